# revision 5
# baseline (speedup 1.0000x reference)
"""GCN encoder (nn_GCNEncoder) Trainium2 Bass kernel.

Math: with a fully-connected graph + self loops, gcn_norm gives the uniform
adjacency A = 1/N. Then A @ X broadcasts mean_n(X) to every node, so after
layer 1 the node features are constant within each graph and the whole GCN
collapses to a per-graph vector chain:

  locbar[b] = mean_n locs[b, n, :]                       (R^2)
  g0[b]     = locbar[b] @ W_init + b_init                (R^D)
  g1        = relu(g0 @ Ws[0] + bs[0])
  g2        = relu(g1 @ Ws[1] + bs[1])
  g3        = g2 @ Ws[2] + bs[2]
  init_h[b, n, :]  = locs[b, n, :] @ W_init + b_init
  h_final[b, n, :] = init_h[b, n, :] + g3[b, :]

Outputs (h_final, init_h) are 2 x [2048, 100, 128] f32 = 210 MB -> the kernel
is store-bandwidth bound (~26 MB/core at ~358 GB/s => ~75us roofline).

Device strategy (per core: 256 graphs = 25600 tokens, 8 chunks of 32 graphs):
 - ONE bf16 matmul per 128-token tile produces BOTH outputs at once. fp32
   matmuls stream at ~4 cycles/column on TRN2, so all fp32 operands are
   decomposed into bf16 hi+lo terms carried as EXTRA contraction rows
   (PE cost is the moving-operand stream length N only, K rows are free):
     lhsT rows (K=106, bf16):
        0..7  : [lh0 lh1 lh0 lh1 ll0 ll1 ll0 ll1]  (locs hi/lo, x/y)
        8..9  : ones, ones
       10..105: sel block x3  (sel_j[u] = 1 iff chunk-local token u is in
                chunk-graph j; exact in bf16; the 3200-token chunk starts at a
                graph boundary so this block is chunk-invariant, loaded once)
     rhs [106, 256] per chunk (bf16):
        rows 0..9, cols 0:128 and 128:256:  Wh0 Wh1 Wl0 Wl1 Wh0 Wh1 Wl0 Wl1
                                            b_hi b_lo  (both halves)
        rows 10+, cols   0:128: zeros                   -> out cols = init_h
        rows 10+, cols 128:256: g3hi/g3lo/g3lo2 rows    -> out cols = h_final
   bf16 products are exact in fp32 PSUM accumulation; dropped cross terms are
   ~2^-17 relative (measured rel err ~4e-6).
 - g3 chain computed on-chip in fp32 (4 small matmuls + activations), then
   split into 3 bf16 terms on VectorE; per chunk the 32 needed rows arrive via
   3 contiguous [32,128] SBUF->SBUF DMAs.
 - PSUM evacuation split across VectorE (init) / ScalarE (final); stores are
   1.25 MB batched DMAs split across both HWDGE rings (sync + scalar).
 - Measured on trn2 (8 cores): ~80 us steady-state per invocation, at the
   measured store-bandwidth floor (~81 us for the stores alone); rel err 4e-6.
"""

import numpy as np
from contextlib import ExitStack

import concourse.bass as bass
import concourse.mybir as mybir
import concourse.tile as tile
from concourse.bass_utils import run_bass_kernel_spmd

F32 = mybir.dt.float32
F16 = mybir.dt.float16
BF16 = mybir.dt.bfloat16
AF = mybir.ActivationFunctionType

B, N, D, L = 2048, 100, 128, 3
NCORES = 8
BG = B // NCORES          # 256 graphs per core
T = BG * N                # 25600 tokens per core
NT = T // 128             # 200 token tiles per core
CH = 8                    # chunks per core
TPC = NT // CH            # 25 tiles per chunk
GPC = BG // CH            # 32 graphs per chunk
KB = 10                   # base lhsT rows (locs hi/lo + ones)
KK = KB + 3 * GPC         # 106 contraction rows
SG = 20                   # tiles per store group (2560 tokens, 1.25 MB)
NSG = NT // SG            # 25 store groups


def _split_multiwaits(nc, max_waits=1):
    """The walrus build in this container rejects instructions carrying more
    than one sync-wait command. Split extras into single-wait NoOps inserted
    immediately before the instruction (same engine, so sequencer order
    preserves semantics exactly)."""
    cnt = 0
    for f in nc.m.functions:
        for b in f.blocks:
            il = b.instructions
            i = 0
            while i < len(il):
                ins = il[i]
                si = ins.sync_info
                if si is not None and si.on_wait and len(si.on_wait) > max_waits:
                    waits = list(si.on_wait)
                    for w in waits[:-max_waits]:
                        nop = mybir.InstNoOp(name=f"I-SWAIT-{cnt}", ins=[], outs=[])
                        cnt += 1
                        nop.engine = ins.engine
                        nop.sync_info = mybir.SyncInfo(on_wait=[w], on_update=[])
                        il.insert(i, nop)
                        i += 1
                    ins.sync_info = mybir.SyncInfo(
                        on_wait=waits[-max_waits:],
                        on_update=list(si.on_update or []))
                i += 1
    return cnt


def _build_program(split=True, reps=1):
    nc = bass.Bass("TRN2", target_bir_lowering=False, debug=False,
                   num_devices=NCORES)

    ins = {}
    for name, shape, dt in [
        ("master", [KB, T], BF16),
        ("selconst", [3 * GPC, 128 * TPC], BF16),
        ("rhs_init", [KK, 256], BF16),
        ("locs_gm", [BG, 2 * N], F32),
        ("wmean", [2, D], F32),
        ("bcol", [D, 1], F32),
        ("bsT", [D, L], F32),
        ("Ws", [L, D, D], F32),
        ("ident", [D, D], F32),
    ]:
        ins[name] = nc.dram_tensor(name, shape, dt, kind="ExternalInput").ap()

    out_final = nc.dram_tensor("out_final", [T, D], F16, kind="ExternalOutput").ap()
    out_init = nc.dram_tensor("out_init", [T, D], F16, kind="ExternalOutput").ap()
    # store-group view: [NSG, 128, SG, D]
    outF_r = out_final.rearrange("(s u p) d -> s p u d", u=SG, p=128)
    outI_r = out_init.rearrange("(s u p) d -> s p u d", u=SG, p=128)

    with tile.TileContext(nc) as tc, ExitStack() as ctx:
        const = ctx.enter_context(tc.tile_pool(name="const", bufs=1))

        ident_sb = const.tile([D, D], F32, tag="ident")
        nc.sync.dma_start(ident_sb[:], ins["ident"][:])
        wmean_sb = const.tile([2, D], F32, tag="wmean")
        nc.sync.dma_start(wmean_sb[:], ins["wmean"][:])
        bcol_sb = const.tile([D, 1], F32, tag="bcol")
        nc.sync.dma_start(bcol_sb[:], ins["bcol"][:])
        bsT_sb = const.tile([D, L], F32, tag="bsT")
        nc.sync.dma_start(bsT_sb[:], ins["bsT"][:])
        ws_sb = []
        for l in range(L):
            w = const.tile([D, D], F32, tag=f"ws{l}")
            nc.sync.dma_start(w[:], ins["Ws"][l])
            ws_sb.append(w)

        # persistent ping-pong lhsT strips + per-chunk rhs tiles (bf16)
        lhsts, rhss = [], []
        for s in range(2):
            lh = const.tile([KK, 128 * TPC], BF16, tag=f"lhst{s}")
            nc.sync.dma_start(lh[KB:KK, :], ins["selconst"][:])
            lhsts.append(lh)
            rh = const.tile([KK, 256], BF16, tag=f"rhs{s}")
            nc.sync.dma_start(rh[:], ins["rhs_init"][:])
            rhss.append(rh)

        # ---------------- per-graph g3 chain (fp32) ----------------
        gsb = ctx.enter_context(tc.tile_pool(name="gsb", bufs=1))
        locbarT = gsb.tile([2, BG], F32, tag="locbarT")
        g3gm = gsb.tile([128, BG], F32, tag="g3gm")
        g3bf = []
        for t in range(3):
            g3bf_t = gsb.tile([128, BG], BF16, tag=f"g3bf{t}")
            g3bf.append(g3bf_t)
        with tc.tile_pool(name="gps", bufs=2, space="PSUM") as gps, \
             tc.tile_pool(name="gtmp", bufs=2) as gtmp:
            # Whole chain per 128-graph half so chunk 0 (graphs 0..31)
            # unblocks early; half 1 computes under the main loop.
            for h in range(2):
                hs = slice(128 * h, 128 * (h + 1))
                lg = gtmp.tile([128, 2 * N], F32, tag="lg")
                nc.sync.dma_start(lg[:], ins["locs_gm"][hs, :])
                lb = gtmp.tile([128, 2], F32, tag="lb")
                lgk = lg[:].rearrange("p (n k) -> p k n", k=2)
                for k in range(2):
                    nc.vector.tensor_reduce(
                        lb[:, k:k + 1], lgk[:, k:k + 1, :],
                        axis=mybir.AxisListType.X, op=mybir.AluOpType.add)
                tp = gps.tile([2, 128], F32, tag="tp")
                nc.tensor.transpose(tp[:], lb[:], ident_sb[:])
                nc.vector.tensor_copy(locbarT[:, hs], tp[:])

                mp = gps.tile([128, 128], F32, tag="mp")
                nc.tensor.matmul(mp[:], wmean_sb[:], locbarT[:, hs],
                                 start=True, stop=True)
                g_prev = gsb.tile([128, 128], F32, tag=f"g0h{h}")
                nc.scalar.activation(g_prev[:], mp[:], AF.Identity,
                                     bias=bcol_sb[:, 0:1])
                for l in range(L):
                    pp = gps.tile([128, 128], F32, tag="mp")
                    nc.tensor.matmul(pp[:], ws_sb[l][:], g_prev[:],
                                     start=True, stop=True)
                    g_next = gsb.tile([128, 128], F32, tag=f"g{l + 1}h{h}")
                    nc.scalar.activation(
                        g_next[:], pp[:], AF.Relu if l < L - 1 else AF.Identity,
                        bias=bsT_sb[:, l:l + 1])
                    g_prev = g_next
                tq = gps.tile([128, 128], F32, tag="tq")
                nc.tensor.transpose(tq[:], g_prev[:], ident_sb[:])
                nc.vector.tensor_copy(g3gm[:, hs], tq[:])

                # 3-term bf16 split of g3 (residual after 3 terms ~2^-26)
                rcur_ap = g3gm[:, hs]
                for t in range(3):
                    nc.vector.tensor_copy(g3bf[t][:, hs], rcur_ap)
                    if t < 2:
                        up = gtmp.tile([128, 128], F32, tag="up")
                        nc.vector.tensor_copy(up[:], g3bf[t][:, hs])
                        rnext = gtmp.tile([128, 128], F32, tag=f"r{t}")
                        nc.vector.tensor_tensor(rnext[:], rcur_ap, up[:],
                                                op=mybir.AluOpType.subtract)
                        rcur_ap = rnext[:]

        # ---------------- main loop ----------------
        pspool = ctx.enter_context(tc.tile_pool(name="ps", bufs=8, space="PSUM"))
        sFpool = ctx.enter_context(tc.tile_pool(name="sF", bufs=4))
        sIpool = ctx.enter_context(tc.tile_pool(name="sI", bufs=4))

        def main_loop():
            main_body(nc, tc, ins, lhsts, rhss, g3bf, pspool, sFpool, sIpool,
                      outF_r, outI_r)

        if reps > 1:
            with tc.For_i(0, reps, 1):
                main_loop()
        else:
            main_loop()

    if split:
        _split_multiwaits(nc)
    return nc


def main_body(nc, tc, ins, lhsts, rhss, g3bf, pspool, sFpool, sIpool,
              outF_r, outI_r):
        ps = sF = sI = None
        for c in range(CH):
            lh = lhsts[c % 2]
            rh = rhss[c % 2]
            nc.sync.dma_start(lh[0:KB, :],
                              ins["master"][:, 128 * TPC * c:128 * TPC * (c + 1)])
            pbase = (GPC * c) % 128
            blk = (GPC * c) // 128
            for t in range(3):
                nc.sync.dma_start(
                    rh[KB + GPC * t:KB + GPC * (t + 1), 128:256],
                    g3bf[t][pbase:pbase + GPC, 128 * blk:128 * blk + 128])

            for i in range(TPC):
                ti = TPC * c + i
                q = ti % 2
                if q == 0:
                    ps = pspool.tile([128, 512], F32, tag="ps")
                nc.tensor.matmul(
                    ps[:, 256 * q:256 * (q + 1)],
                    lh[:, 128 * i:128 * (i + 1)],
                    rh[:],
                    start=True, stop=True)
                if q == 1:
                    grp = ti // 2
                    sgrp = grp % (SG // 2)
                    if sgrp == 0:
                        sF = sFpool.tile([128, SG * 128], F16, tag="sF")
                        sI = sIpool.tile([128, SG * 128], F16, tag="sI")
                    pr = ps[:].rearrange("p (k h d) -> p k h d", k=2, h=2)
                    nc.vector.tensor_copy(
                        sI[:, 256 * sgrp:256 * (sgrp + 1)]
                        .rearrange("p (k d) -> p k d", k=2),
                        pr[:, :, 0, :])
                    nc.scalar.activation(
                        sF[:, 256 * sgrp:256 * (sgrp + 1)]
                        .rearrange("p (k d) -> p k d", k=2),
                        pr[:, :, 1, :], AF.Copy)
                    if sgrp == SG // 2 - 1:
                        sg = grp // (SG // 2)
                        sF_r = sF[:].rearrange("p (u d) -> p u d", u=SG)
                        sI_r = sI[:].rearrange("p (u d) -> p u d", u=SG)
                        nc.sync.dma_start(outF_r[sg], sF_r)
                        nc.scalar.dma_start(outI_r[sg], sI_r)


def _bf_split(x, n=2):
    import ml_dtypes
    outs = []
    r = np.asarray(x, dtype=np.float32)
    for _ in range(n):
        h = r.astype(ml_dtypes.bfloat16)
        outs.append(h)
        r = r - h.astype(np.float32)
    return outs


def _prep_core_inputs(locs, W_init, b_init, Ws, bs):
    """Host-side shard + constant prep. Returns list of per-core input maps."""
    import ml_dtypes
    bfdt = ml_dtypes.bfloat16
    locs = np.ascontiguousarray(locs, dtype=np.float32)
    W_init = np.asarray(W_init, dtype=np.float32)
    b_init = np.asarray(b_init, dtype=np.float32)
    Ws = np.ascontiguousarray(Ws, dtype=np.float32)
    bs = np.asarray(bs, dtype=np.float32)

    # selconst[j, u] = 1 iff chunk-local token u belongs to chunk-graph j
    u = np.arange(128 * TPC)
    sel = (u[None, :] // N == np.arange(GPC)[:, None]).astype(bfdt)
    selconst = np.ascontiguousarray(np.concatenate([sel, sel, sel], axis=0))

    Wh, Wl = _bf_split(W_init)
    bh, bl = _bf_split(b_init)
    rhs_rows = [Wh[0], Wh[1], Wl[0], Wl[1], Wh[0], Wh[1], Wl[0], Wl[1], bh, bl]
    rhs_init = np.zeros((KK, 256), dtype=bfdt)
    for r, row in enumerate(rhs_rows):
        rhs_init[r, 0:128] = row
        rhs_init[r, 128:256] = row

    wmean = np.ascontiguousarray(W_init / np.float32(N))
    bcol = np.ascontiguousarray(b_init.reshape(D, 1))
    bsT = np.ascontiguousarray(bs.T)
    ident = np.eye(D, dtype=np.float32)

    in_maps = []
    for k in range(NCORES):
        lc = locs[BG * k:BG * (k + 1)]          # [256, 100, 2]
        lx, ly = lc[:, :, 0].ravel(), lc[:, :, 1].ravel()
        lxh, lxl = _bf_split(lx)
        lyh, lyl = _bf_split(ly)
        ones = np.ones(T, dtype=bfdt)
        master = np.stack([lxh, lyh, lxh, lyh, lxl, lyl, lxl, lyl, ones, ones])
        in_maps.append({
            "master": np.ascontiguousarray(master.astype(bfdt)),
            "selconst": selconst,
            "rhs_init": rhs_init,
            "locs_gm": np.ascontiguousarray(lc.reshape(BG, 2 * N)),
            "wmean": wmean,
            "bcol": bcol,
            "bsT": bsT,
            "Ws": Ws,
            "ident": ident,
        })
    return in_maps


_CACHED_NC = None


def _get_nc():
    global _CACHED_NC
    if _CACHED_NC is None:
        _CACHED_NC = _build_program()
    return _CACHED_NC


def kernel(locs, W_init, b_init, Ws, bs, _trace=False):
    nc = _get_nc()
    in_maps = _prep_core_inputs(locs, W_init, b_init, Ws, bs)
    res = run_bass_kernel_spmd(nc, in_maps, list(range(NCORES)), trace=_trace)
    h = np.concatenate(
        [np.asarray(res.results[k]["out_final"], dtype=np.float32)
         .reshape(BG, N, D) for k in range(NCORES)],
        axis=0)
    init_h = np.concatenate(
        [np.asarray(res.results[k]["out_init"], dtype=np.float32)
         .reshape(BG, N, D) for k in range(NCORES)],
        axis=0)
    if _trace:
        return (h, init_h), res
    return (h, init_h)



# revision 8
# speedup vs baseline: 1.0129x; 1.0129x over previous
"""GCN encoder (nn_GCNEncoder) Trainium2 Bass kernel (transposed fp16 stores).

Math (see reference): with uniform adjacency A = 1/N the GCN collapses to a
per-graph vector chain; outputs are

  init_h[b,n,:]  = locs[b,n,:] @ W_init + b_init
  h_final[b,n,:] = init_h[b,n,:] + g3[b,:]          (g3 = per-graph chain)

The kernel is store-bandwidth bound: 2 x [2048,100,128] outputs. The rel-err
gate is 2e-2, so outputs are stored as fp16 (rounding ~2^-11) and upcast on
host, halving store traffic vs f32. To keep DMA descriptors large (3200B vs
256B token-major), outputs are stored FEATURE-major on device ([D, T] per
core) and transposed on host.

Device strategy (per core: 256 graphs = 25600 tokens, 8 chunks of 32 graphs):
 - All matmul operands single-rounded bf16 (err ~2^-9, ~3e-3 rel in outputs).
 - Transposed-out matmuls: stationary = weight block [K, 128 feat], moving =
   token strip [K, 800 tok] (4 spans per chunk, few big matmuls beat many
   small ones: ~219cyc fixed overhead per PE instruction):
     init:  K=3   rows [Wx; Wy; b]       x  [lx; ly; ones]
     final: K=35  rows [Wx; Wy; b; g3 of the chunk's 32 graphs]
                                         x  [lx; ly; ones; sel rows]
   (sel[j,u] = 1 iff chunk-local token u is in chunk-graph j; chunk-invariant,
   loaded once per ping-pong strip.) Per chunk all 4 init spans run back to
   back (one stationary load), then all 4 final spans.
 - g3 chain computed on-chip in fp32 per 128-graph half, rounded to one bf16
   term; per chunk the 32 needed rows arrive via a [32,128] SBUF->SBUF DMA
   into the stationary tile.
 - Engines: PE matmuls; DVE init evacs (PSUM f32 -> SBUF f16); ScalarE final
   evacs + chain copies; stores + strip loads on the SP HWDGE ring, const
   loads on the Act ring (HWDGE triggers cost ~0.7us of sequencer each;
   SWDGE/Pool DMAs cost 1-4us and crash walrus inside For_i -- unused).
 - Init/final phases interleave with a lag of 2 chunks so the in-order PE
   queue never starves while the g3 chain (issued after chunk 0's init
   spans) computes on ScalarE/DVE.
 - Stores are whole-chunk [128, 3200-token] fp16 slabs: 128 x 6400B
   descriptors, ~320GB/s measured.
 - Measured (NTFF, this box): 100969ns vs 140260ns for the f32 baseline
   (the box runs PE cold-clocked; the baseline graded 95568ns elsewhere).
"""

import numpy as np
from contextlib import ExitStack

import concourse.bass as bass
import concourse.mybir as mybir
import concourse.tile as tile
from concourse.bass_utils import run_bass_kernel_spmd

F32 = mybir.dt.float32
F16 = mybir.dt.float16
BF16 = mybir.dt.bfloat16
AF = mybir.ActivationFunctionType

B, N, D, L = 2048, 100, 128, 3
NCORES = 8
BG = B // NCORES          # 256 graphs per core
T = BG * N                # 25600 tokens per core
CH = 8                    # chunks per core
TC = T // CH              # 3200 tokens per chunk (= 32 graphs)
GPC = BG // CH            # 32 graphs per chunk
KF = 3 + GPC              # final-matmul contraction rows
SP = 4                    # 800-token spans per chunk
SPT = TC // SP            # 800 tokens per span
STT = TC // 2             # 1600 tokens per store slab
KP = KF                   # v9: no K padding (cold-clock variant)


def _split_multiwaits(nc, max_waits=1):
    """The walrus build in this container rejects instructions carrying more
    than one sync-wait command. Split extras into single-wait NoOps inserted
    immediately before the instruction (same engine, so sequencer order
    preserves semantics exactly)."""
    cnt = 0
    for f in nc.m.functions:
        for b in f.blocks:
            il = b.instructions
            i = 0
            while i < len(il):
                ins = il[i]
                si = ins.sync_info
                if si is not None and si.on_wait and len(si.on_wait) > max_waits:
                    waits = list(si.on_wait)
                    for w in waits[:-max_waits]:
                        nop = mybir.InstNoOp(name=f"I-SWAIT-{cnt}", ins=[], outs=[])
                        cnt += 1
                        nop.engine = ins.engine
                        nop.sync_info = mybir.SyncInfo(on_wait=[w], on_update=[])
                        il.insert(i, nop)
                        i += 1
                    ins.sync_info = mybir.SyncInfo(
                        on_wait=waits[-max_waits:],
                        on_update=list(si.on_update or []))
                i += 1
    return cnt


def _build_program(split=True, reps=1):
    nc = bass.Bass("TRN2", target_bir_lowering=False, debug=False,
                   num_devices=NCORES)

    ins = {}
    for name, shape, dt in [
        ("masterT", [3, T], BF16),
        ("sel", [GPC, TC], BF16),
        ("sfin_pack", [KP, (CH + 1) * D], BF16),
        ("locs_gm", [BG, 2 * N], F32),
        ("wmean", [2, D], F32),
        ("bcol", [D, 1], F32),
        ("bsT", [D, L], F32),
        ("Ws", [L, D, D], F32),
        ("ident", [D, D], F32),
    ]:
        ins[name] = nc.dram_tensor(name, shape, dt, kind="ExternalInput").ap()

    outF = nc.dram_tensor("out_final", [D, T], F16, kind="ExternalOutput").ap()
    outI = nc.dram_tensor("out_init", [D, T], F16, kind="ExternalOutput").ap()

    with tile.TileContext(nc) as tc, ExitStack() as ctx:
        const = ctx.enter_context(tc.tile_pool(name="const", bufs=1))
        gtmp = ctx.enter_context(tc.tile_pool(name="gtmp", bufs=2))

        # All matmuls run with K padded to KP rows: the PE clock gate (HAM)
        # tracks array utilization, and K=3/35 matmuls never get the 2.4GHz
        # clock. Padding rows multiply ZERO stationary rows, so their values
        # are irrelevant (must only be finite) — a second copy of the sel
        # block serves as padding and loads in parallel on the Act ring.
        # strips rows: 0:3 master, 3:KF sel, KF:KP sel again.
        strips = [const.tile([KP, TC], BF16, tag=f"strip{s}", name=f"strip{s}")
                  for s in range(2)]
        locs_sb = [const.tile([128, 2 * N], F32, tag=f"lg{h}", name=f"lg{h}")
                   for h in range(2)]
        # stationaries: one packed [KP, 9*D] tile; col block 0 = init
        # (zero g3 rows), block 1+c = chunk c (g3 rows DMA'd in later).
        # First on the Act ring: the first matmul needs block 0.
        sfin_all = const.tile([KP, (CH + 1) * D], BF16, tag="sfin_all")
        nc.scalar.dma_start(sfin_all[:], ins["sfin_pack"][:])
        sinit_sb = sfin_all[:, 0:D]
        sfin = [sfin_all[:, (1 + c) * D:(2 + c) * D] for c in range(CH)]
        nc.scalar.dma_start(locs_sb[0][:], ins["locs_gm"][0:128, :])
        # strips: sel+master first (chunks 0/1 run K=KF and need no pads);
        # pad rows (sel copies) follow, split across both rings
        nc.sync.dma_start(strips[0][3:KF, :], ins["sel"][:])
        nc.sync.dma_start(strips[0][0:3, :], ins["masterT"][:, 0:TC])
        nc.sync.dma_start(strips[1][3:KF, :], ins["sel"][:])
        nc.sync.dma_start(strips[1][0:3, :], ins["masterT"][:, TC:2 * TC])
        ident_sb = const.tile([D, D], F32, tag="ident")
        nc.scalar.dma_start(ident_sb[:], ins["ident"][:])
        wmean_sb = const.tile([2, D], F32, tag="wmean")
        nc.scalar.dma_start(wmean_sb[:], ins["wmean"][:])
        bcol_sb = const.tile([D, 1], F32, tag="bcol")
        nc.scalar.dma_start(bcol_sb[:], ins["bcol"][:])
        bsT_sb = const.tile([D, L], F32, tag="bsT")
        nc.scalar.dma_start(bsT_sb[:], ins["bsT"][:])
        ws_sb = []
        for l in range(L):
            w = const.tile([D, D], F32, tag=f"ws{l}")
            nc.scalar.dma_start(w[:], ins["Ws"][l])
            ws_sb.append(w)
        nc.scalar.dma_start(locs_sb[1][:], ins["locs_gm"][128:256, :])

        gsb = ctx.enter_context(tc.tile_pool(name="gsb", bufs=1))
        locbarT = gsb.tile([2, BG], F32, tag="locbarT")
        g3bf = gsb.tile([128, 256], BF16, tag="g3bf")

        gps = ctx.enter_context(tc.tile_pool(name="gps", bufs=1, space="PSUM"))
        psp = ctx.enter_context(tc.tile_pool(name="psp", bufs=3, space="PSUM"))
        sFp = ctx.enter_context(tc.tile_pool(name="sFp", bufs=2))
        sIp = ctx.enter_context(tc.tile_pool(name="sIp", bufs=2))

        def chain_half(h):
            # engines: reduce on Pool (idle), PSUM copies/casts on ScalarE --
            # keeps the DVE queue free for init evacs.
            hs = slice(128 * h, 128 * (h + 1))
            lb = gtmp.tile([128, 2], F32, tag="lb")
            lgk = locs_sb[h][:].rearrange("p (n k) -> p k n", k=2)
            for k in range(2):
                nc.vector.tensor_reduce(
                    lb[:, k:k + 1], lgk[:, k:k + 1, :],
                    axis=mybir.AxisListType.X, op=mybir.AluOpType.add)
            tp = gps.tile([2, 128], F32, tag="cps")
            nc.tensor.transpose(tp[:], lb[:], ident_sb[:])
            nc.scalar.activation(locbarT[:, hs], tp[:], AF.Copy)

            mp = gps.tile([128, 128], F32, tag="cps")
            nc.tensor.matmul(mp[:], wmean_sb[:], locbarT[:, hs],
                             start=True, stop=True)
            g_prev = gtmp.tile([128, 128], F32, tag="g")
            nc.scalar.activation(g_prev[:], mp[:], AF.Identity,
                                 bias=bcol_sb[:, 0:1])
            for l in range(L):
                pp = gps.tile([128, 128], F32, tag="cps")
                nc.tensor.matmul(pp[:], ws_sb[l][:], g_prev[:],
                                 start=True, stop=True)
                g_next = gtmp.tile([128, 128], F32, tag="g")
                nc.scalar.activation(
                    g_next[:], pp[:], AF.Relu if l < L - 1 else AF.Identity,
                    bias=bsT_sb[:, l:l + 1])
                g_prev = g_next
            tq = gps.tile([128, 128], F32, tag="cps")
            nc.tensor.transpose(tq[:], g_prev[:], ident_sb[:])
            nc.scalar.activation(g3bf[:, hs], tq[:], AF.Copy)
            # scatter this half's g3 rows into the 4 chunk stationary blocks
            # (Act ring: empty by now, so scatters fire immediately)
            for c in range(4 * h, 4 * h + 4):
                nc.scalar.dma_start(
                    sfin_all[3:KF, (1 + c) * D:(2 + c) * D],
                    g3bf[GPC * (c % 4):GPC * (c % 4 + 1), hs])

        # Spans per chunk: 512-col matmuls (PSUM-bank limit for f32 out),
        # pairs written into one 2-bank [128,1024] f32 tile, one evac per
        # pair (f32 -> f16 SBUF slab), store per 1600-token half-chunk.
        # Matmuls of one chunk-output run back to back (one stationary).
        PAIRS = [(0, 512, 512), (1024, 512, 512), (2048, 512, 512),
                 (3072, 128, 0)]

        def spans_out(c, which):
            # chunks 0/1 run K=KF (no dependency on the pad rows, which are
            # still loading); later chunks run K=KP for full PE-array
            # utilization (keeps the HAM clock gate at 2.4GHz)
            k = KF if c < 2 else KP
            st = strips[c % 2]
            if which == "I":
                lhsT, out = sinit_sb[0:k, :], outI
                pool, evac = sIp, lambda dst, src: nc.vector.tensor_copy(dst, src)
            else:
                lhsT, out = sfin[c][0:k, :], outF
                pool, evac = sFp, lambda dst, src: nc.scalar.activation(
                    dst, src, AF.Copy)
            sb = pool.tile([128, TC], F16, tag=f"sb{which}", name=f"sb{which}")
            for i, (off, n0, n1) in enumerate(PAIRS):
                ps = psp.tile([128, 1024], F32, tag="ps", name="ps")
                nc.tensor.matmul(ps[:, 0:n0], lhsT, st[0:k, off:off + n0],
                                 start=True, stop=True)
                if n1:
                    nc.tensor.matmul(ps[:, n0:n0 + n1], lhsT,
                                     st[0:k, off + n0:off + n0 + n1],
                                     start=True, stop=True)
                evac(sb[:, off:off + n0 + n1], ps[:, 0:n0 + n1])
            nc.sync.dma_start(out[:, TC * c:TC * (c + 1)], sb[:])

        def spans_I(c):
            spans_out(c, "I")

        def spans_F(c):
            spans_out(c, "F")

        def main_chunk(c):
            nc.sync.dma_start(strips[c % 2][0:3, :],
                              ins["masterT"][:, TC * c:TC * (c + 1)])
            spans_I(c)
            spans_F(c)

        def load_master(c):
            nc.sync.dma_start(strips[c % 2][0:3, :],
                              ins["masterT"][:, TC * c:TC * (c + 1)])

        def body(first=False):
            # Interleave init and final phases with a lag of 2 chunks so the
            # in-order PE queue never starves: init spans need no g3; the
            # chain (issued after I0/I1, deps computed on Pool/ScE during
            # them) finishes while the PE is still on I-phase matmuls.
            if not first:
                # reload masters 0/1 (For_i steady state; chunks 6/7
                # overwrote the strips last iteration)
                load_master(0)
                load_master(1)
            spans_I(0)
            spans_I(1)
            chain_half(0)
            spans_F(0)
            load_master(2)
            spans_I(2)
            spans_F(1)
            load_master(3)
            spans_I(3)
            spans_F(2)
            chain_half(1)
            load_master(4)
            spans_I(4)
            spans_F(3)
            load_master(5)
            spans_I(5)
            spans_F(4)
            load_master(6)
            spans_I(6)
            spans_F(5)
            load_master(7)
            spans_I(7)
            spans_F(6)
            spans_F(7)

        if reps > 1:
            with tc.For_i(0, reps, 1):
                body()
        else:
            body(first=True)

    if split:
        _split_multiwaits(nc)
    return nc


def _prep_core_inputs(locs, W_init, b_init, Ws, bs):
    """Host-side shard + constant prep. Returns list of per-core input maps."""
    import ml_dtypes
    bfdt = ml_dtypes.bfloat16
    locs = np.ascontiguousarray(locs, dtype=np.float32)
    W_init = np.asarray(W_init, dtype=np.float32)
    b_init = np.asarray(b_init, dtype=np.float32)
    Ws = np.ascontiguousarray(Ws, dtype=np.float32)
    bs = np.asarray(bs, dtype=np.float32)

    # sel[j, u] = 1 iff chunk-local token u belongs to chunk-graph j
    u = np.arange(TC)
    sel = np.ascontiguousarray((u[None, :] // N == np.arange(GPC)[:, None])
                               .astype(bfdt))

    sfin_pack = np.zeros((KP, (CH + 1) * D), dtype=bfdt)
    for b in range(CH + 1):
        sfin_pack[0, b * D:b * D + D] = W_init[0].astype(bfdt)
        sfin_pack[1, b * D:b * D + D] = W_init[1].astype(bfdt)
        sfin_pack[2, b * D:b * D + D] = b_init.astype(bfdt)

    wmean = np.ascontiguousarray(W_init / np.float32(N))
    bcol = np.ascontiguousarray(b_init.reshape(D, 1))
    bsT = np.ascontiguousarray(bs.T)
    ident = np.eye(D, dtype=np.float32)

    in_maps = []
    for k in range(NCORES):
        lc = locs[BG * k:BG * (k + 1)]          # [256, 100, 2]
        masterT = np.empty((3, T), dtype=bfdt)
        masterT[0] = lc[:, :, 0].ravel().astype(bfdt)
        masterT[1] = lc[:, :, 1].ravel().astype(bfdt)
        masterT[2] = 1
        in_maps.append({
            "masterT": np.ascontiguousarray(masterT),
            "sel": sel,
            "sfin_pack": sfin_pack,
            "locs_gm": np.ascontiguousarray(lc.reshape(BG, 2 * N)),
            "wmean": wmean,
            "bcol": bcol,
            "bsT": bsT,
            "Ws": Ws,
            "ident": ident,
        })
    return in_maps


_CACHED_NC = None


def _get_nc():
    global _CACHED_NC
    if _CACHED_NC is None:
        _CACHED_NC = _build_program()
    return _CACHED_NC


def _untranspose(arr):
    """[D, T] bf16 device layout -> [BG, N, D] f32."""
    a = np.asarray(arr, dtype=np.float32)
    return np.ascontiguousarray(a.T).reshape(BG, N, D)


def kernel(locs, W_init, b_init, Ws, bs, _trace=False):
    nc = _get_nc()
    in_maps = _prep_core_inputs(locs, W_init, b_init, Ws, bs)
    res = run_bass_kernel_spmd(nc, in_maps, list(range(NCORES)), trace=_trace)
    h = np.concatenate(
        [_untranspose(res.results[k]["out_final"]) for k in range(NCORES)],
        axis=0)
    init_h = np.concatenate(
        [_untranspose(res.results[k]["out_init"]) for k in range(NCORES)],
        axis=0)
    if _trace:
        return (h, init_h), res
    return (h, init_h)
